# revision 46
# baseline (speedup 1.0000x reference)
"""NoisyHadamardLinear Trainium2 kernel (self-contained).

y = blockwise_FHT_1024(x) @ W^T + b  for x [2, 4096, 4096], W [4096, 4096],
b [4096], on 8 NeuronCores, data-parallel over the 8192 tokens (1024/core).

The blockwise Hadamard is a symmetric orthogonal map, so it is folded into
the weights on the host:  y = x @ (W Hb)^T + b  with W' = blockwise_FHT(W)
computed once in numpy. The host packs x and W' into the exact SBUF layouts
the device needs, and the device runs a pure GEMM entirely with fp8
DoubleRow matmuls (0.5 PE cycles/row, two 128-deep contraction groups per
instruction) using two-word fp8 arithmetic:

  value = Hi (e4m3) + Lo (e5m2 residual);   x @ w ~= Xh@Wh + Xh@Wl + Xl@Wh

12 of the 16 contraction pair-groups compute all three terms (near-fp16
accuracy, only the ~0.13% Xl@Wl term is dropped), one computes Xh@Wh +
Xh@Wl, and the last 3 use the hi-words only — the op-count-optimal mix
for the error budget. Hi words carry a power-of-2 split scale (x*2^-3,
W'*2^3) so products land unscaled in the shared fp32 PSUM and both
operands sit in e4m3's normal range; lo words reuse the same scales in
e5m2's wide exponent range. A host-side subnormal policy (round to
nearest of {0, +-min_normal}) keeps the shipped bytes bit-deterministic
whether or not the PE flushes fp8 subnormals. Hardware-measured max rel
err 1.790e-2 vs the 2e-2 gate on the fixed seed-0 inputs, matching the
host-side numpy prediction to ~2e-5 (as did every prior config).

Schedule: for the first o-slab the pair loop is OUTER (8 concurrent PSUM
chains), with per-pair W/x tiles interleaved in the DMA stream so the PE
starts ~4us in and streams right behind the DMA (tri-pair wire cost
~2.2us < 2.56us of matmuls per step; the cheap mono pairs go last).
Remaining slabs run chain-at-a-time from one big W-slab DMA each (2KB+
lines avoid the ~656 ns per-DMA wire quantum). Hi and lo W words share
one e4m3 dram tensor; the lo slices are bitcast to e5m2 at use. Eviction
adds the bias (per-partition scalar) on the ACT engine, which also issues
the per-chunk output DMAs; the final chain splits its eviction across
DVE+ACT and ships from the SP queue to shorten the tail. Host transposes
YT back.
"""
import numpy as np

import concourse.bacc as bacc
import concourse.mybir as mybir
import concourse.tile as tile
from concourse.bass_utils import run_bass_kernel_spmd

P = 128
f16 = mybir.dt.float16
f32 = mybir.dt.float32
e4 = mybir.dt.float8e4
e5 = mybir.dt.float8e5

N_CORES = 8
B, S, D, O = 2, 4096, 4096, 4096
HAD_BLOCK = 1024
T_PER_CORE = (B * S) // N_CORES   # 1024 tokens per core

NPAIR = 16                         # pair-groups of 256 contraction dims
NTRI = 12                          # pairs with both lo cross terms
NBI = 1                            # pairs with the W-lo term only
NLO = NTRI + NBI                   # pairs carrying a W lo word-block (13)
NMONO = NPAIR - NLO                # hi-word-only pairs (the last 3)
XSCALE = np.float32(2.0 ** -3)     # host scale on x hi/lo words
WSCALE = np.float32(2.0 ** 3)      # host scale on W' hi/lo words
OS = 512                           # o-slab width
NOS = O // OS                      # 8 o-slabs
NOT = OS // P                      # 4 o-tiles (128 rows) per slab
TCH = 512                          # t-chunk (PSUM free dim)
WPB = 2 * OS                       # bytes per W word-block per pair row
WROW = (NPAIR + NLO) * WPB         # W bytes per slab row (hi then lo)


def build_kernel(T=T_PER_CORE, num_devices=N_CORES):
    NTC = T // TCH                 # t-chunks per core (2)
    DR = mybir.MatmulPerfMode.DoubleRow

    nc = bacc.Bacc("TRN2", target_bir_lowering=False, debug=False,
                   num_devices=num_devices, dynamic_dma_scratch_size=2048)
    # x hi words, per-pair tiles: XH[j*128+p, k*T+t] =
    #   e4m3(xT[(2j+k)*128+p, t] * XSCALE)
    XH = nc.dram_tensor("XH", [NPAIR * P, 2 * T], e4, kind="ExternalInput")
    # x lo words (e5m2 residuals, same scale), tri pairs only
    XL = nc.dram_tensor("XL", [NTRI * P, 2 * T], e5, kind="ExternalInput")
    # Head pack: pair-0 W hi word-block + pair-0 x hi words — one DMA
    # (one sem) covers the first matmul's operands; pair-0's W lo block
    # rides separately (only the third op of each chain needs it).
    HP = nc.dram_tensor("HP", [P, WPB + 2 * T], e4, kind="ExternalInput")
    # W slabs 0 and 1, per-pair combined (hi|lo) word-blocks:
    # rows (os*NPAIR+j)*128+p, cols [0:WPB) hi (k*OS+o), [WPB:2*WPB) lo
    WP0 = nc.dram_tensor("WP0", [2 * NPAIR * P, 2 * WPB], e4,
                         kind="ExternalInput")
    # W slabs 2..NOS-1: one row-block per slab; cols: NPAIR hi word-blocks
    # then NTRI lo word-blocks (bytes; lo slices bitcast to e5m2 at use)
    WPS = nc.dram_tensor("WPS", [(NOS - 2) * P, WROW], e4,
                         kind="ExternalInput")
    # bias packed per o-tile column: BP[p, j] = b[j*128 + p]
    BP = nc.dram_tensor("BP", [P, NOS * NOT], f32, kind="ExternalInput")
    # y^T: [o, t] fp32 (host transposes back)
    YT = nc.dram_tensor("YT", [O, T], f32, kind="ExternalOutput")

    with tile.TileContext(nc) as tc:
        with tc.tile_pool(name="const", bufs=1) as cpool, \
             tc.tile_pool(name="xh", bufs=NPAIR) as xhp, \
             tc.tile_pool(name="xl", bufs=NTRI) as xlp, \
             tc.tile_pool(name="w0", bufs=2 * NPAIR) as w0p, \
             tc.tile_pool(name="ws", bufs=2) as wsp, \
             tc.tile_pool(name="yo", bufs=8) as yop, \
             tc.tile_pool(name="ps", bufs=8, space="PSUM") as psp:
            # PE p-state warmup: ~2.5us of scratch matmuls keep the PE
            # continuously busy (and the frequency ramp climbing) until the
            # first real operands land.
            scr = cpool.tile([P, 512], f16)
            nc.vector.memset(scr[:], 0.0)
            wps = psp.tile([16, 512], f32, tag="ps", name="warmup")
            for i in range(5):
                nc.tensor.matmul(wps[:], scr[:, 0:16], scr[:],
                                 start=True, stop=True)

            # Slab 0 per-pair W and x tiles, interleaved: the pair-outer
            # first slab streams right behind this DMA order. Pair 0 comes
            # from the combined head pack (one DMA, one sem).
            head = cpool.tile([P, WPB + 2 * T], e4)
            nc.sync.dma_start(head[:], HP.ap())
            xh = [head[:, WPB:]]              # pair-0 x hi words
            wl0 = cpool.tile([P, WPB], e4)    # pair-0 W lo word-block
            nc.sync.dma_start(wl0[:], WP0.ap()[0:P, WPB:2 * WPB])
            w0 = [None]                       # pair 0 handled via head/wl0
            xl = []
            xl0 = xlp.tile([P, 2 * T], e5, tag="xl", name="xl0")
            nc.sync.dma_start(xl0[:], XL.ap()[0:P, :])
            xl.append(xl0[:])
            for j in range(1, NPAIR):
                w_ = w0p.tile([P, 2 * WPB], e4, tag="w0", name=f"w0_{j}")
                nc.sync.dma_start(w_[:], WP0.ap()[j * P:(j + 1) * P, :])
                w0.append(w_[:])
                xh_ = xhp.tile([P, 2 * T], e4, tag="xh", name=f"xh{j}")
                nc.sync.dma_start(xh_[:], XH.ap()[j * P:(j + 1) * P, :])
                xh.append(xh_[:])
                if j < NTRI:
                    xl_ = xlp.tile([P, 2 * T], e5, tag="xl", name=f"xl{j}")
                    nc.sync.dma_start(xl_[:], XL.ap()[j * P:(j + 1) * P, :])
                    xl.append(xl_[:])

            bt = cpool.tile([P, NOS * NOT], f32)
            nc.sync.dma_start(bt[:], BP.ap())

            # Second warmup batch gated on the head pack (the first DMA):
            # bridges any remaining idle gap before the real matmuls.
            wps2 = psp.tile([16, 16], f32, tag="ps", name="warmup2")
            for i in range(4):
                nc.tensor.matmul(wps2[:], head[0:P, 0:16], head[0:P, 0:16],
                                 start=True, stop=True)

            # Slab 1 per-pair W tiles: stream right behind slab 0's (their
            # wire time doesn't fit ahead of slab 1 as one big DMA).
            w1 = []
            for j in range(NPAIR):
                w_ = w0p.tile([P, 2 * WPB], e4, tag="w0", name=f"w1_{j}")
                r = (NPAIR + j) * P
                nc.sync.dma_start(w_[:], WP0.ap()[r:r + P, :])
                w1.append(w_[:])

            def load_w_slab(os_):
                t_ = wsp.tile([P, WROW], e4, tag="ws", name=f"ws{os_}")
                nc.sync.dma_start(
                    t_[:], WPS.ap()[(os_ - 2) * P:(os_ - 1) * P, :])
                return t_

            wts = {2: load_w_slab(2)}

            def wh_l(ap_block, ot):
                """lhsT [128, 2, 128] from a hi/lo word-block [P, 2*OS]."""
                return ap_block.rearrange(
                    "p (k o) -> p k o", k=2)[:, :, ot * P:(ot + 1) * P]

            def xrhs(ap_words, tc_):
                """rhs [128, 2, TCH] from x words [P, 2*T]."""
                return ap_words.rearrange(
                    "p (k t) -> p k t", k=2)[:, :, tc_ * TCH:(tc_ + 1) * TCH]

            def evict_out(os_, ot, tc_, py):
                # PSUM eviction with fused bias add (per-partition scalar)
                # on the ACT engine, which also issues the output DMA.
                yo = yop.tile([P, TCH], f32, tag="yo")
                nc.scalar.add(yo[:], py[:],
                              bt[:, os_ * NOT + ot: os_ * NOT + ot + 1])
                orow = os_ * OS + ot * P
                nc.scalar.dma_start(
                    YT.ap()[orow:orow + P, tc_ * TCH:(tc_ + 1) * TCH], yo[:])

            def pair_matmuls(py, whi, wlo, j, ot, tc_, first, last):
                """Emit the DR matmuls of pair j into PSUM tile py."""
                # hi @ hi
                nc.tensor.matmul(py, wh_l(whi, ot), xrhs(xh[j], tc_),
                                 start=first, stop=(last and j >= NLO),
                                 perf_mode=DR)
                if j < NTRI:
                    # hi(W) @ lo(x)
                    nc.tensor.matmul(py, wh_l(whi, ot), xrhs(xl[j], tc_),
                                     start=False, stop=False, perf_mode=DR)
                if j < NLO:
                    # lo(W) @ hi(x)
                    nc.tensor.matmul(py, wh_l(wlo, ot).bitcast(e5),
                                     xrhs(xh[j], tc_),
                                     start=False, stop=last, perf_mode=DR)

            def slab_w_blocks(os_, wt, j):
                """(hi, lo) word-block APs for pair j of slab os_."""
                if os_ <= 1:
                    if os_ == 0 and j == 0:
                        return head[:, 0:WPB], wl0[:]
                    blk = w0[j] if os_ == 0 else w1[j]
                    return blk[:, 0:WPB], blk[:, WPB:2 * WPB]
                hi = wt[:, j * WPB:(j + 1) * WPB]
                if j >= NLO:
                    return hi, None
                lo = wt[:, (NPAIR + j) * WPB:(NPAIR + j + 1) * WPB]
                return hi, lo

            # Slabs 0 and 1: pair-outer, 8 concurrent PSUM chains stream
            # behind the DMA. Tri pairs 0..NTRI-1 first, mono pairs last.
            for os_ in (0, 1):
                pys = [psp.tile([P, TCH], f32, tag="ps",
                                name=f"ps{os_}_{j}")
                       for j in range(NOT * NTC)]
                for j in range(NPAIR):
                    whi, wlo = slab_w_blocks(os_, None, j)
                    order = ([(ot, tc_) for tc_ in range(NTC)
                              for ot in range(NOT)]
                             if (os_, j) == (0, 0) else
                             [(ot, tc_) for ot in range(NOT)
                              for tc_ in range(NTC)])
                    for ot, tc_ in order:
                        pair_matmuls(pys[ot * NTC + tc_][:], whi, wlo, j,
                                     ot, tc_, first=(j == 0),
                                     last=(j == NPAIR - 1))
                for ot in range(NOT):
                    for tc_ in range(NTC):
                        evict_out(os_, ot, tc_, pys[ot * NTC + tc_])

            # Slabs 2..NOS-1: chain-at-a-time, everything resident.
            for os_ in range(2, NOS):
                wt = wts.pop(os_)
                if os_ + 1 < NOS:
                    wts[os_ + 1] = load_w_slab(os_ + 1)
                for ot in range(NOT):
                    for tc_ in range(NTC):
                        py = psp.tile([P, TCH], f32, tag="ps")
                        for j in range(NPAIR):
                            whi, wlo = slab_w_blocks(os_, wt, j)
                            pair_matmuls(py[:], whi, wlo, j, ot, tc_,
                                         first=(j == 0),
                                         last=(j == NPAIR - 1))
                        last = (os_ == NOS - 1 and ot == NOT - 1
                                and tc_ == NTC - 1)
                        if not last:
                            evict_out(os_, ot, tc_, py)
                        else:
                            # Tail: split the final eviction across DVE and
                            # ACT concurrently, DMA from the (idle) SP queue.
                            bcol = bt[:, os_ * NOT + ot: os_ * NOT + ot + 1]
                            yo = yop.tile([P, TCH], f32, tag="yo")
                            half = TCH // 2
                            nc.vector.tensor_scalar_add(
                                yo[:, 0:half], py[:, 0:half], bcol)
                            nc.scalar.add(
                                yo[:, half:TCH], py[:, half:TCH], bcol)
                            orow = os_ * OS + ot * P
                            nc.sync.dma_start(
                                YT.ap()[orow:orow + P,
                                        tc_ * TCH:(tc_ + 1) * TCH], yo[:])
    nc.compile()
    return nc


_CACHED_NC = None


def _get_nc():
    global _CACHED_NC
    if _CACHED_NC is None:
        _CACHED_NC = build_kernel()
    return _CACHED_NC


def _q_safe(a, dt, mn):
    """Quantize with no subnormals in the result: subnormal-range values
    round to the nearest of {0, +-mn}. The shipped bytes then decode
    identically whether or not the PE flushes fp8 subnormals."""
    q = a.astype(dt).astype(np.float32)
    small = np.abs(q) < mn
    q = np.where(small, np.where(np.abs(a) >= mn / 2,
                                 (np.sign(a) * mn).astype(np.float32),
                                 np.float32(0.0)), q)
    return q


def _fwht_rows(a):
    """Unnormalized fast Walsh-Hadamard transform along axis 1."""
    m, n = a.shape
    h = 1
    while h < n:
        a = a.reshape(m, n // (2 * h), 2, h)
        s = a[:, :, 0, :] + a[:, :, 1, :]
        d = a[:, :, 0, :] - a[:, :, 1, :]
        a = np.stack([s, d], axis=2).reshape(m, n)
        h *= 2
    return a


def kernel(x, W, b):
    x = np.asarray(x, dtype=np.float32)
    W = np.asarray(W, dtype=np.float32)
    b = np.asarray(b, dtype=np.float32)
    assert x.shape == (B, S, D) and W.shape == (O, D) and b.shape == (O,)

    nc = _get_nc()
    np4 = mybir.dt.np(e4)
    np5 = mybir.dt.np(e5)
    mn4 = np.float32(2.0 ** -6)
    mn5 = np.float32(2.0 ** -14)

    # Fold the blockwise Hadamard into W:  y = x @ (W Hb)^T + b
    Wf = _fwht_rows(W.reshape(-1, HAD_BLOCK)).reshape(O, D)
    Wf *= np.float32(1.0 / np.sqrt(HAD_BLOCK))
    WfT = np.ascontiguousarray(Wf.T)  # [d, o]
    # Two-word quantization of W' (scaled by WSCALE)
    Whq = _q_safe(WfT * WSCALE, np4, mn4)            # hi values (scaled)
    Wh8 = Whq.astype(np4)                            # e4m3 bytes
    Wlq = _q_safe((WfT * WSCALE) - Whq, np5, mn5)    # lo residual (scaled)
    Wl8 = Wlq.astype(np5)                            # e5m2 bytes

    def w_block(arr8, j, os_):
        """[P, 2*OS] word-block bytes: rows of pair j, slab os_ columns."""
        blk = arr8[2 * j * P:(2 * j + 2) * P, os_ * OS:(os_ + 1) * OS]
        # [2, P, OS] -> [P, 2*OS] with k-major columns
        return np.ascontiguousarray(
            blk.reshape(2, P, OS).transpose(1, 0, 2).reshape(P, 2 * OS))

    u8 = np.uint8
    # Slabs 0 and 1: per-pair combined (hi|lo) blocks
    WP0h = np.concatenate(
        [np.concatenate([w_block(Wh8, j, os_).view(u8),
                         w_block(Wl8, j, os_).view(u8)], axis=1)
         for os_ in (0, 1) for j in range(NPAIR)], axis=0)
    # Slabs 2..: per slab, NPAIR hi blocks then NTRI lo blocks
    WPSh = np.concatenate(
        [np.concatenate(
            [w_block(Wh8, j, os_).view(u8) for j in range(NPAIR)]
            + [w_block(Wl8, j, os_).view(u8) for j in range(NLO)], axis=1)
         for os_ in range(2, NOS)], axis=0)
    BP = np.ascontiguousarray(b.reshape(NOS * NOT, P).T)

    xf = x.reshape(B * S, D)
    in_maps = []
    for c in range(N_CORES):
        XTfull = np.ascontiguousarray(
            xf[c * T_PER_CORE:(c + 1) * T_PER_CORE].T)   # [d, t] fp32
        Xhq = _q_safe(XTfull * XSCALE, np4, mn4)
        Xh8 = Xhq.astype(np4)
        Xl8 = _q_safe((XTfull * XSCALE) - Xhq, np5, mn5).astype(np5)

        def x_words(arr8, j):
            blk = arr8[2 * j * P:(2 * j + 2) * P, :]     # [2*P, T]
            return np.ascontiguousarray(
                blk.reshape(2, P, T_PER_CORE).transpose(1, 0, 2)
                .reshape(P, 2 * T_PER_CORE))

        XHc = np.concatenate(
            [x_words(Xh8, j).view(u8) for j in range(NPAIR)], axis=0)
        XLc = np.concatenate(
            [x_words(Xl8, j).view(u8) for j in range(NTRI)], axis=0)
        HPc = np.concatenate(
            [w_block(Wh8, 0, 0).view(u8), x_words(Xh8, 0).view(u8)], axis=1)
        in_maps.append({
            "XH": XHc.view(np4), "XL": XLc.view(np5),
            "WP0": WP0h.view(np4), "WPS": WPSh.view(np4),
            "BP": BP, "HP": np.ascontiguousarray(HPc).view(np4)})
    res = run_bass_kernel_spmd(nc, in_maps, core_ids=list(range(N_CORES)))
    y = np.concatenate(
        [np.ascontiguousarray(res.results[c]["YT"].T)
         for c in range(N_CORES)], axis=0)
    return y.reshape(B, S, O).astype(np.float32, copy=False)


# revision 50
# speedup vs baseline: 1.0050x; 1.0050x over previous
"""NoisyHadamardLinear Trainium2 kernel (self-contained).

y = blockwise_FHT_1024(x) @ W^T + b  for x [2, 4096, 4096], W [4096, 4096],
b [4096], on 8 NeuronCores, data-parallel over the 8192 tokens (1024/core).

The blockwise Hadamard is a symmetric orthogonal map, so it is folded into
the weights on the host:  y = x @ (W Hb)^T + b  with W' = blockwise_FHT(W)
computed once in numpy. The host packs x and W' into the exact SBUF layouts
the device needs, and the device runs a pure GEMM entirely with fp8
DoubleRow matmuls (0.5 PE cycles/row, two 128-deep contraction groups per
instruction) using two-word fp8 arithmetic:

  value = Hi (e4m3) + Lo (e5m2 residual);   x @ w ~= Xh@Wh + Xh@Wl + Xl@Wh

12 of the 16 contraction pair-groups compute all three terms (near-fp16
accuracy, only the ~0.13% Xl@Wl term is dropped), one computes Xh@Wh +
Xh@Wl, and the last 3 use the hi-words only — the op-count-optimal mix
for the error budget. Hi words carry a power-of-2 split scale (x*2^-3,
W'*2^3) so products land unscaled in the shared fp32 PSUM and both
operands sit in e4m3's normal range; lo words reuse the same scales in
e5m2's wide exponent range. A host-side subnormal policy (round to
nearest of {0, +-min_normal}) keeps the shipped bytes bit-deterministic
whether or not the PE flushes fp8 subnormals. Hardware-measured max rel
err 1.790e-2 vs the 2e-2 gate on the fixed seed-0 inputs, matching the
host-side numpy prediction to ~2e-5 (as did every prior config).

Schedule: for the first o-slab the pair loop is OUTER (8 concurrent PSUM
chains), with per-pair W/x tiles interleaved in the DMA stream so the PE
starts ~4us in and streams right behind the DMA (tri-pair wire cost
~2.2us < 2.56us of matmuls per step; the cheap mono pairs go last).
Remaining slabs run chain-at-a-time from one big W-slab DMA each (2KB+
lines avoid the ~656 ns per-DMA wire quantum). Hi and lo W words share
one e4m3 dram tensor; the lo slices are bitcast to e5m2 at use. Eviction
adds the bias (per-partition scalar) on the ACT engine, which also issues
the per-chunk output DMAs; the final chain splits its eviction across
DVE+ACT and ships from the SP queue to shorten the tail. Host transposes
YT back.
"""
import numpy as np

import concourse.bacc as bacc
import concourse.mybir as mybir
import concourse.tile as tile
from concourse.bass_utils import run_bass_kernel_spmd

P = 128
f16 = mybir.dt.float16
f32 = mybir.dt.float32
e4 = mybir.dt.float8e4
e5 = mybir.dt.float8e5

N_CORES = 8
B, S, D, O = 2, 4096, 4096, 4096
HAD_BLOCK = 1024
T_PER_CORE = (B * S) // N_CORES   # 1024 tokens per core

NPAIR = 16                         # pair-groups of 256 contraction dims
NTRI = 12                          # pairs with both lo cross terms
NBI = 1                            # pairs with the W-lo term only
NLO = NTRI + NBI                   # pairs carrying a W lo word-block (13)
NMONO = NPAIR - NLO                # hi-word-only pairs (the last 3)
XSCALE = np.float32(2.0 ** -3)     # host scale on x hi/lo words
WSCALE = np.float32(2.0 ** 3)      # host scale on W' hi/lo words
OS = 512                           # o-slab width
NOS = O // OS                      # 8 o-slabs
NOT = OS // P                      # 4 o-tiles (128 rows) per slab
TCH = 512                          # t-chunk (PSUM free dim)
WPB = 2 * OS                       # bytes per W word-block per pair row
WROW = (NPAIR + NLO) * WPB         # W bytes per slab row (hi then lo)


def build_kernel(T=T_PER_CORE, num_devices=N_CORES):
    NTC = T // TCH                 # t-chunks per core (2)
    DR = mybir.MatmulPerfMode.DoubleRow

    nc = bacc.Bacc("TRN2", target_bir_lowering=False, debug=False,
                   num_devices=num_devices, dynamic_dma_scratch_size=2048)
    # x hi words, per-pair tiles: XH[j*128+p, k*T+t] =
    #   e4m3(xT[(2j+k)*128+p, t] * XSCALE)
    XH = nc.dram_tensor("XH", [NPAIR * P, 2 * T], e4, kind="ExternalInput")
    # x lo words (e5m2 residuals, same scale), tri pairs only
    XL = nc.dram_tensor("XL", [NTRI * P, 2 * T], e5, kind="ExternalInput")
    # Head pack: pair-0 W hi word-block + pair-0 x hi words — one DMA
    # (one sem) covers the first matmul's operands; pair-0's W lo block
    # rides separately (only the third op of each chain needs it).
    HP = nc.dram_tensor("HP", [P, WPB + 2 * T], e4, kind="ExternalInput")
    # W slabs 0 and 1, per-pair combined (hi|lo) word-blocks:
    # rows (os*NPAIR+j)*128+p, cols [0:WPB) hi (k*OS+o), [WPB:2*WPB) lo
    WP0 = nc.dram_tensor("WP0", [2 * NPAIR * P, 2 * WPB], e4,
                         kind="ExternalInput")
    # W slabs 2..NOS-1: one row-block per slab; cols: NPAIR hi word-blocks
    # then NTRI lo word-blocks (bytes; lo slices bitcast to e5m2 at use)
    WPS = nc.dram_tensor("WPS", [(NOS - 2) * P, WROW], e4,
                         kind="ExternalInput")
    # bias packed per o-tile column: BP[p, j] = b[j*128 + p]
    BP = nc.dram_tensor("BP", [P, NOS * NOT], f32, kind="ExternalInput")
    # y^T: [o, t] fp32 (host transposes back)
    YT = nc.dram_tensor("YT", [O, T], f32, kind="ExternalOutput")

    with tile.TileContext(nc) as tc:
        with tc.tile_pool(name="const", bufs=1) as cpool, \
             tc.tile_pool(name="xh", bufs=NPAIR) as xhp, \
             tc.tile_pool(name="xl", bufs=NTRI) as xlp, \
             tc.tile_pool(name="w0", bufs=2 * NPAIR) as w0p, \
             tc.tile_pool(name="ws", bufs=2) as wsp, \
             tc.tile_pool(name="yo", bufs=8) as yop, \
             tc.tile_pool(name="ps", bufs=8, space="PSUM") as psp:
            # PE p-state warmup: ~2.5us of scratch matmuls keep the PE
            # continuously busy (and the frequency ramp climbing) until the
            # first real operands land.
            scr = cpool.tile([P, 512], f16)
            nc.vector.memset(scr[:], 0.0)
            wps = psp.tile([16, 512], f32, tag="ps", name="warmup")
            for i in range(5):
                nc.tensor.matmul(wps[:], scr[:, 0:16], scr[:],
                                 start=True, stop=True)

            # Slab 0 per-pair W and x tiles, interleaved: the pair-outer
            # first slab streams right behind this DMA order. Pair 0 comes
            # from the combined head pack (one DMA, one sem).
            head = cpool.tile([P, WPB + 2 * T], e4)
            nc.sync.dma_start(head[:], HP.ap())
            xh = [head[:, WPB:]]              # pair-0 x hi words
            wl0 = cpool.tile([P, WPB], e4)    # pair-0 W lo word-block
            nc.sync.dma_start(wl0[:], WP0.ap()[0:P, WPB:2 * WPB])
            w0 = [None]                       # pair 0 handled via head/wl0
            xl = []
            xl0 = xlp.tile([P, 2 * T], e5, tag="xl", name="xl0")
            nc.sync.dma_start(xl0[:], XL.ap()[0:P, :])
            xl.append(xl0[:])
            for j in range(1, NPAIR):
                w_ = w0p.tile([P, 2 * WPB], e4, tag="w0", name=f"w0_{j}")
                nc.sync.dma_start(w_[:], WP0.ap()[j * P:(j + 1) * P, :])
                w0.append(w_[:])
                xh_ = xhp.tile([P, 2 * T], e4, tag="xh", name=f"xh{j}")
                nc.sync.dma_start(xh_[:], XH.ap()[j * P:(j + 1) * P, :])
                xh.append(xh_[:])
                if j < NTRI:
                    xl_ = xlp.tile([P, 2 * T], e5, tag="xl", name=f"xl{j}")
                    nc.sync.dma_start(xl_[:], XL.ap()[j * P:(j + 1) * P, :])
                    xl.append(xl_[:])

            bt = cpool.tile([P, NOS * NOT], f32)
            nc.sync.dma_start(bt[:], BP.ap())

            # Second warmup batch gated on the head pack (the first DMA):
            # bridges any remaining idle gap before the real matmuls.
            wps2 = psp.tile([16, 16], f32, tag="ps", name="warmup2")
            for i in range(4):
                nc.tensor.matmul(wps2[:], head[0:P, 0:16], head[0:P, 0:16],
                                 start=True, stop=True)

            # Slab 1 per-pair W tiles: stream right behind slab 0's (their
            # wire time doesn't fit ahead of slab 1 as one big DMA).
            w1 = []
            for j in range(NPAIR):
                w_ = w0p.tile([P, 2 * WPB], e4, tag="w0", name=f"w1_{j}")
                r = (NPAIR + j) * P
                nc.sync.dma_start(w_[:], WP0.ap()[r:r + P, :])
                w1.append(w_[:])

            def load_w_slab(os_):
                t_ = wsp.tile([P, WROW], e4, tag="ws", name=f"ws{os_}")
                nc.sync.dma_start(
                    t_[:], WPS.ap()[(os_ - 2) * P:(os_ - 1) * P, :])
                return t_

            wts = {2: load_w_slab(2)}

            def wh_l(ap_block, ot):
                """lhsT [128, 2, 128] from a hi/lo word-block [P, 2*OS]."""
                return ap_block.rearrange(
                    "p (k o) -> p k o", k=2)[:, :, ot * P:(ot + 1) * P]

            def xrhs(ap_words, t0, tw):
                """rhs [128, 2, tw] from x words [P, 2*T]."""
                return ap_words.rearrange(
                    "p (k t) -> p k t", k=2)[:, :, t0:t0 + tw]

            def evict_out(os_, ot, t0, tw, py):
                # PSUM eviction with fused bias add (per-partition scalar)
                # on the ACT engine, which also issues the output DMA.
                yo = yop.tile([P, tw], f32, tag="yo")
                nc.scalar.add(yo[:], py[:],
                              bt[:, os_ * NOT + ot: os_ * NOT + ot + 1])
                orow = os_ * OS + ot * P
                nc.scalar.dma_start(
                    YT.ap()[orow:orow + P, t0:t0 + tw], yo[:])

            def pair_matmuls(py, whi, wlo, j, ot, t0, tw, first, last):
                """Emit the DR matmuls of pair j into PSUM tile py."""
                # hi @ hi
                nc.tensor.matmul(py, wh_l(whi, ot), xrhs(xh[j], t0, tw),
                                 start=first, stop=(last and j >= NLO),
                                 perf_mode=DR)
                if j < NTRI:
                    # hi(W) @ lo(x)
                    nc.tensor.matmul(py, wh_l(whi, ot), xrhs(xl[j], t0, tw),
                                     start=False, stop=False, perf_mode=DR)
                if j < NLO:
                    # lo(W) @ hi(x)
                    nc.tensor.matmul(py, wh_l(wlo, ot).bitcast(e5),
                                     xrhs(xh[j], t0, tw),
                                     start=False, stop=last, perf_mode=DR)

            def slab_w_blocks(os_, wt, j):
                """(hi, lo) word-block APs for pair j of slab os_."""
                if os_ <= 1:
                    if os_ == 0 and j == 0:
                        return head[:, 0:WPB], wl0[:]
                    blk = w0[j] if os_ == 0 else w1[j]
                    return blk[:, 0:WPB], blk[:, WPB:2 * WPB]
                hi = wt[:, j * WPB:(j + 1) * WPB]
                if j >= NLO:
                    return hi, None
                lo = wt[:, (NPAIR + j) * WPB:(NPAIR + j + 1) * WPB]
                return hi, lo

            # Slabs 0 and 1: pair-outer, 8 concurrent PSUM chains stream
            # behind the DMA. Tri pairs 0..NTRI-1 first, mono pairs last.
            for os_ in (0, 1):
                pys = [psp.tile([P, TCH], f32, tag="ps",
                                name=f"ps{os_}_{j}")
                       for j in range(NOT * NTC)]
                for j in range(NPAIR):
                    whi, wlo = slab_w_blocks(os_, None, j)
                    order = ([(ot, tc_) for tc_ in range(NTC)
                              for ot in range(NOT)]
                             if (os_, j) == (0, 0) else
                             [(ot, tc_) for ot in range(NOT)
                              for tc_ in range(NTC)])
                    for ot, tc_ in order:
                        pair_matmuls(pys[ot * NTC + tc_][:], whi, wlo, j,
                                     ot, tc_ * TCH, TCH, first=(j == 0),
                                     last=(j == NPAIR - 1))
                for ot in range(NOT):
                    for tc_ in range(NTC):
                        evict_out(os_, ot, tc_ * TCH, TCH,
                                  pys[ot * NTC + tc_])

            # Slabs 2..NOS-1: chain-at-a-time, everything resident.
            for os_ in range(2, NOS):
                wt = wts.pop(os_)
                if os_ + 1 < NOS:
                    wts[os_ + 1] = load_w_slab(os_ + 1)
                for ot in range(NOT):
                    for t0, tw in ((0, 504), (504, 504), (1008, 16)):
                        py = psp.tile([P, tw], f32, tag="ps")
                        for j in range(NPAIR):
                            whi, wlo = slab_w_blocks(os_, wt, j)
                            pair_matmuls(py[:], whi, wlo, j, ot, t0, tw,
                                         first=(j == 0),
                                         last=(j == NPAIR - 1))
                        last = (os_ == NOS - 1 and ot == NOT - 1
                                and t0 == 1008)
                        if not last:
                            evict_out(os_, ot, t0, tw, py)
                        else:
                            # Tail: the final chunk is only 16 tokens —
                            # tiny eviction, DMA from the (idle) SP queue.
                            bcol = bt[:, os_ * NOT + ot: os_ * NOT + ot + 1]
                            yo = yop.tile([P, tw], f32, tag="yo")
                            nc.scalar.add(yo[:], py[:], bcol)
                            orow = os_ * OS + ot * P
                            nc.sync.dma_start(
                                YT.ap()[orow:orow + P, t0:t0 + tw], yo[:])
    nc.compile()
    return nc


_CACHED_NC = None


def _get_nc():
    global _CACHED_NC
    if _CACHED_NC is None:
        _CACHED_NC = build_kernel()
    return _CACHED_NC


def _q_safe(a, dt, mn):
    """Quantize with no subnormals in the result: subnormal-range values
    round to the nearest of {0, +-mn}. The shipped bytes then decode
    identically whether or not the PE flushes fp8 subnormals."""
    q = a.astype(dt).astype(np.float32)
    small = np.abs(q) < mn
    q = np.where(small, np.where(np.abs(a) >= mn / 2,
                                 (np.sign(a) * mn).astype(np.float32),
                                 np.float32(0.0)), q)
    return q


def _fwht_rows(a):
    """Unnormalized fast Walsh-Hadamard transform along axis 1."""
    m, n = a.shape
    h = 1
    while h < n:
        a = a.reshape(m, n // (2 * h), 2, h)
        s = a[:, :, 0, :] + a[:, :, 1, :]
        d = a[:, :, 0, :] - a[:, :, 1, :]
        a = np.stack([s, d], axis=2).reshape(m, n)
        h *= 2
    return a


def kernel(x, W, b):
    x = np.asarray(x, dtype=np.float32)
    W = np.asarray(W, dtype=np.float32)
    b = np.asarray(b, dtype=np.float32)
    assert x.shape == (B, S, D) and W.shape == (O, D) and b.shape == (O,)

    nc = _get_nc()
    np4 = mybir.dt.np(e4)
    np5 = mybir.dt.np(e5)
    mn4 = np.float32(2.0 ** -6)
    mn5 = np.float32(2.0 ** -14)

    # Fold the blockwise Hadamard into W:  y = x @ (W Hb)^T + b
    Wf = _fwht_rows(W.reshape(-1, HAD_BLOCK)).reshape(O, D)
    Wf *= np.float32(1.0 / np.sqrt(HAD_BLOCK))
    WfT = np.ascontiguousarray(Wf.T)  # [d, o]
    # Two-word quantization of W' (scaled by WSCALE)
    Whq = _q_safe(WfT * WSCALE, np4, mn4)            # hi values (scaled)
    Wh8 = Whq.astype(np4)                            # e4m3 bytes
    Wlq = _q_safe((WfT * WSCALE) - Whq, np5, mn5)    # lo residual (scaled)
    Wl8 = Wlq.astype(np5)                            # e5m2 bytes

    def w_block(arr8, j, os_):
        """[P, 2*OS] word-block bytes: rows of pair j, slab os_ columns."""
        blk = arr8[2 * j * P:(2 * j + 2) * P, os_ * OS:(os_ + 1) * OS]
        # [2, P, OS] -> [P, 2*OS] with k-major columns
        return np.ascontiguousarray(
            blk.reshape(2, P, OS).transpose(1, 0, 2).reshape(P, 2 * OS))

    u8 = np.uint8
    # Slabs 0 and 1: per-pair combined (hi|lo) blocks
    WP0h = np.concatenate(
        [np.concatenate([w_block(Wh8, j, os_).view(u8),
                         w_block(Wl8, j, os_).view(u8)], axis=1)
         for os_ in (0, 1) for j in range(NPAIR)], axis=0)
    # Slabs 2..: per slab, NPAIR hi blocks then NTRI lo blocks
    WPSh = np.concatenate(
        [np.concatenate(
            [w_block(Wh8, j, os_).view(u8) for j in range(NPAIR)]
            + [w_block(Wl8, j, os_).view(u8) for j in range(NLO)], axis=1)
         for os_ in range(2, NOS)], axis=0)
    BP = np.ascontiguousarray(b.reshape(NOS * NOT, P).T)

    xf = x.reshape(B * S, D)
    in_maps = []
    for c in range(N_CORES):
        XTfull = np.ascontiguousarray(
            xf[c * T_PER_CORE:(c + 1) * T_PER_CORE].T)   # [d, t] fp32
        Xhq = _q_safe(XTfull * XSCALE, np4, mn4)
        Xh8 = Xhq.astype(np4)
        Xl8 = _q_safe((XTfull * XSCALE) - Xhq, np5, mn5).astype(np5)

        def x_words(arr8, j):
            blk = arr8[2 * j * P:(2 * j + 2) * P, :]     # [2*P, T]
            return np.ascontiguousarray(
                blk.reshape(2, P, T_PER_CORE).transpose(1, 0, 2)
                .reshape(P, 2 * T_PER_CORE))

        XHc = np.concatenate(
            [x_words(Xh8, j).view(u8) for j in range(NPAIR)], axis=0)
        XLc = np.concatenate(
            [x_words(Xl8, j).view(u8) for j in range(NTRI)], axis=0)
        HPc = np.concatenate(
            [w_block(Wh8, 0, 0).view(u8), x_words(Xh8, 0).view(u8)], axis=1)
        in_maps.append({
            "XH": XHc.view(np4), "XL": XLc.view(np5),
            "WP0": WP0h.view(np4), "WPS": WPSh.view(np4),
            "BP": BP, "HP": np.ascontiguousarray(HPc).view(np4)})
    res = run_bass_kernel_spmd(nc, in_maps, core_ids=list(range(N_CORES)))
    y = np.concatenate(
        [np.ascontiguousarray(res.results[c]["YT"].T)
         for c in range(N_CORES)], axis=0)
    return y.reshape(B, S, O).astype(np.float32, copy=False)
